# revision 14
# baseline (speedup 1.0000x reference)
"""Trainium2 Bass kernel for MatrixFactorizationIF (embedding-lookup style).

Computation per batch element b with indices (i, j, k):
    pFi = pF[i]   # [448] = [Pi(64) | Vs(192) | Vg(192)]
    out[b] = ALPHA * <Pi, M[j]>
           + BETA^2 * sum_s ( <Vs[:, s], M[j]> * <Vg[:, s], M[k]> )

v3 strategy (evolved from the shipped baseline):
  - All tables bf16: pF shard rows are 1024B gather descriptors (512
    bf16 cols incl. pad; elem_size must be %256B, >=512B avoids the DMA
    read-modify-write penalty, vs 1792B in f32); M rows are
    stored duplicated [m|m] as 256B bf16 rows so the gather stays legal
    (elem_size % 256B) and the compute needs no f32->bf16 converts.
  - pF columns host-permuted s-major so Vs/Vg DVE reads are stride-1
    (2x bf16 DVE rate needs contiguous access).
  - Batch sharded by i-range (12500 pF rows per core, i_loc < 12500 fits
    int16 with no grouping).  M gathers need int16 indices, so elements
    are grouped by (j%4, k%4): 16 groups padded to 4096 (seed-0 max group
    count is 4070; overflow raises) with j_loc, k_loc = j>>2, k>>2.
  - One gather call per tensor per group (single_packet=False lifts the
    1024-index single-packet cap), calls round-robined over 4 SWDGE
    queues so descriptor generation parallelizes across Q7 cpu pairs.
  - All int16 indices loaded in one big upfront DMA; outputs stored
    p-major so every store is contiguous per partition.
"""

import numpy as np

N_P = 100000
N_M = 100000
R = 64
S = 3
E = R * (1 + 2 * S)  # 448
EP = 512             # padded bf16 row (1024B, must be %256B)
B = 500000
ALPHA = 0.001
BETA = 0.001

N_CORES = 8
P = 128
PF_SHARD = N_P // N_CORES   # 12500 pF rows per core
NJ = 4                      # j stride classes
NK = 4                      # k stride classes
NG = NJ * NK                # 16 groups per core
GCAP = 4096                 # padded group capacity (32 * 128)
GCOLS = GCAP // 16          # 256 wrapped-index columns
TCOL = GCAP // P            # 33
BS_PAD = NG * GCAP          # 65536 padded elements per core
NQ = 4                      # SWDGE queues


def build_program(repeat=1):
    import concourse.bass as bass
    import concourse.bacc as bacc
    import concourse.mybir as mybir
    from concourse.tile import TileContext

    f32 = mybir.dt.float32
    bf16 = mybir.dt.bfloat16
    i16 = mybir.dt.int16
    mult = mybir.AluOpType.mult
    add = mybir.AluOpType.add
    AX = mybir.AxisListType.X
    t = TCOL

    nc = bacc.Bacc("TRN2", target_bir_lowering=False, num_swdge_queues=NQ)
    pFs = nc.dram_tensor("pFs", [PF_SHARD, EP], bf16, kind="ExternalInput")
    Md = nc.dram_tensor("Md", [N_M, 2 * R], bf16, kind="ExternalInput")
    idx = nc.dram_tensor("idx", [P, NG, 3 * GCOLS], i16, kind="ExternalInput")
    out = nc.dram_tensor("out", [BS_PAD], f32, kind="ExternalOutput")

    with TileContext(nc) as tc:
        with (
            tc.tile_pool(name="idxp", bufs=1) as idx_pool,
            tc.tile_pool(name="pf", bufs=2) as pf_pool,
            tc.tile_pool(name="m", bufs=3) as m_pool,
            tc.tile_pool(name="prod", bufs=2) as prod_pool,
            tc.tile_pool(name="small", bufs=2) as small_pool,
            tc.tile_pool(name="res", bufs=3) as res_pool,
        ):
            idx_t = idx_pool.tile([P, NG * 3 * GCOLS], i16)
            nc.sync.dma_start(
                out=idx_t[:].rearrange("p (n q) -> p n q", q=3 * GCOLS),
                in_=idx[:])
            idx3 = idx_t[:].rearrange("p (n q) -> p n q", q=3 * GCOLS)

            qn = 0
            for it in range(NG * repeat):
                g = it % NG
                jc, kc = g // NK, g % NK
                c0 = g * GCAP

                pf_t = pf_pool.tile([P, t * EP], bf16)
                mj_t = m_pool.tile([P, t * 2 * R], bf16, tag="mj")
                mk_t = m_pool.tile([P, t * 2 * R], bf16, tag="mk")
                pf4 = pf_t[:].rearrange("p (t e) -> p t e", e=EP)
                mj3 = mj_t[:].rearrange("p (t d) -> p t d", d=2 * R)
                mk3 = mk_t[:].rearrange("p (t d) -> p t d", d=2 * R)

                mjview = Md[:].rearrange(
                    "(n f) d -> n (f d)", f=NJ)[:, jc * 2 * R:(jc + 1) * 2 * R]
                mkview = Md[:].rearrange(
                    "(n f) d -> n (f d)", f=NK)[:, kc * 2 * R:(kc + 1) * 2 * R]

                nc.gpsimd.dma_gather(
                    out_ap=pf4,
                    in_ap=pFs[:],
                    idxs_ap=idx3[:, g, 0:GCOLS],
                    num_idxs=GCAP, num_idxs_reg=GCAP, elem_size=EP,
                    single_packet=False, queue_num=qn % NQ)
                nc.gpsimd.dma_gather(
                    out_ap=mj3,
                    in_ap=mjview,
                    idxs_ap=idx3[:, g, GCOLS:2 * GCOLS],
                    num_idxs=GCAP, num_idxs_reg=GCAP, elem_size=2 * R,
                    elem_step=2 * R * NJ,
                    single_packet=False, queue_num=(qn + 1) % NQ)
                nc.gpsimd.dma_gather(
                    out_ap=mk3,
                    in_ap=mkview,
                    idxs_ap=idx3[:, g, 2 * GCOLS:3 * GCOLS],
                    num_idxs=GCAP, num_idxs_reg=GCAP, elem_size=2 * R,
                    elem_step=2 * R * NK,
                    single_packet=False, queue_num=(qn + 2) % NQ)
                qn += 3

                mj = mj3[:, :, 0:R]
                mk = mk3[:, :, 0:R]

                # dot = sum_r Pi * Mj
                prod1 = prod_pool.tile([P, t * R], bf16, tag="prod1")
                nc.vector.tensor_tensor(
                    out=prod1[:].rearrange("p (t r) -> p t r", r=R),
                    in0=pf4[:, :, 0:R], in1=mj, op=mult)
                dot = small_pool.tile([P, t], f32, tag="dot")
                nc.vector.reduce_sum(
                    out=dot[:],
                    in_=prod1[:].rearrange("p (t r) -> p t r", r=R), axis=AX)

                # a[t, s] = sum_r Vs[t, s, r] * Mj[t, r]  (s-major layout)
                vs_view = pf4[:, :, R:(1 + S) * R].rearrange(
                    "p t (s r) -> p t s r", s=S)
                mjb = mj[:, :, None, :].to_broadcast([P, t, S, R])
                pvs = prod_pool.tile([P, t * S * R], bf16, tag="pvs")
                nc.vector.tensor_tensor(
                    out=pvs[:].rearrange("p (t s r) -> p t s r", s=S, r=R),
                    in0=vs_view, in1=mjb, op=mult)
                a_t = small_pool.tile([P, t * S], f32, tag="a")
                nc.vector.reduce_sum(
                    out=a_t[:],
                    in_=pvs[:].rearrange("p (t s r) -> p t s r", s=S, r=R),
                    axis=AX)

                # g[t, s] = sum_r Vg[t, s, r] * Mk[t, r]
                vg_view = pf4[:, :, (1 + S) * R:E].rearrange(
                    "p t (s r) -> p t s r", s=S)
                mkb = mk[:, :, None, :].to_broadcast([P, t, S, R])
                pvg = prod_pool.tile([P, t * S * R], bf16, tag="pvg")
                nc.vector.tensor_tensor(
                    out=pvg[:].rearrange("p (t s r) -> p t s r", s=S, r=R),
                    in0=vg_view, in1=mkb, op=mult)
                g_t = small_pool.tile([P, t * S], f32, tag="g")
                nc.vector.reduce_sum(
                    out=g_t[:],
                    in_=pvg[:].rearrange("p (t s r) -> p t s r", s=S, r=R),
                    axis=AX)

                # agdot = sum_s a*g ; res = ALPHA*dot + BETA^2*agdot
                agp = small_pool.tile([P, t * S], f32, tag="agp")
                nc.vector.tensor_mul(out=agp[:], in0=a_t[:], in1=g_t[:])
                agdot = small_pool.tile([P, t], f32, tag="agdot")
                nc.vector.reduce_sum(
                    out=agdot[:],
                    in_=agp[:].rearrange("p (t s) -> p t s", s=S), axis=AX)
                agdot_b = small_pool.tile([P, t], f32, tag="agdot_b")
                nc.vector.tensor_scalar_mul(
                    out=agdot_b[:], in0=agdot[:], scalar1=BETA * BETA)
                res = res_pool.tile([P, t], f32, tag="res")
                nc.vector.scalar_tensor_tensor(
                    out=res[:], in0=dot[:], scalar=ALPHA, in1=agdot_b[:],
                    op0=mult, op1=add)

                # element rank r in group -> slot g*GCAP + (r%128)*33 + r//128
                nc.sync.dma_start(
                    out=out[c0:c0 + GCAP].rearrange("(p t) -> p t", p=P),
                    in_=res[:])

    nc.compile()
    return nc


_NC_CACHE = {}


def _get_program():
    if "main" not in _NC_CACHE:
        _NC_CACHE["main"] = build_program()
    return _NC_CACHE["main"]


def _to_bf16(x):
    import ml_dtypes
    return x.astype(ml_dtypes.bfloat16)


def prepare_inputs(pF, M, ijk):
    """Host-side table conversion + shard/sort/pad of indices.  Returns
    (in_maps, src_index): src_index[b] is the flat position of original
    element b in the concatenated per-core padded outputs."""
    # permute pF columns: Vs/Vg from r-major (s fastest) to s-major
    vs = pF[:, R:(1 + S) * R].reshape(N_P, R, S)
    vg = pF[:, (1 + S) * R:].reshape(N_P, R, S)
    pFp = np.concatenate([
        pF[:, :R],
        np.ascontiguousarray(vs.transpose(0, 2, 1)).reshape(N_P, S * R),
        np.ascontiguousarray(vg.transpose(0, 2, 1)).reshape(N_P, S * R),
        np.zeros((N_P, EP - E), np.float32),
    ], axis=1)
    pFb = np.ascontiguousarray(_to_bf16(pFp))
    Mdup = np.ascontiguousarray(
        np.repeat(_to_bf16(M), 2, axis=0).reshape(N_M, 2 * R))

    i = ijk[:, 0].astype(np.int64)
    j = ijk[:, 1].astype(np.int64)
    k0 = ijk[:, 2].astype(np.int64)
    k = np.where(k0 < 0, 0, k0)

    core = i // PF_SHARD
    gl = (j % NJ) * NK + (k % NK)            # group within core
    gg = core * NG + gl                      # global group id
    i_loc_key = i - core * PF_SHARD
    order = np.argsort(gg * 16384 + i_loc_key, kind="stable")
    counts = np.bincount(gg, minlength=N_CORES * NG)
    if counts.max() > GCAP:
        raise RuntimeError(
            f"group overflow: max {counts.max()} > {GCAP}")
    starts = np.zeros(N_CORES * NG, np.int64)
    starts[1:] = np.cumsum(counts)[:-1]
    rank = np.arange(B) - np.repeat(starts, counts)
    rank_orig = np.empty(B, np.int64)
    rank_orig[order] = rank
    # store layout: slot = g*GCAP + (rank%128)*TCOL + rank//128
    src_index = (core * BS_PAD + gl * GCAP
                 + (rank_orig % P) * TCOL + rank_orig // P)

    i_loc = (i - core * PF_SHARD).astype(np.int16)
    j_loc = (j >> 2).astype(np.int16)
    k_loc = (k >> 2).astype(np.int16)

    # wrapped idx layout: element rank e -> [e % 16, e // 16]
    wrapped = np.zeros((N_CORES, NG, 3, 16, GCOLS), np.int16)
    wp = (rank_orig % 16).astype(np.int64)
    ws = (rank_orig // 16).astype(np.int64)
    wrapped[core, gl, 0, wp, ws] = i_loc
    wrapped[core, gl, 1, wp, ws] = j_loc
    wrapped[core, gl, 2, wp, ws] = k_loc
    # replicate the 16-partition wrap to all 128 partitions; partition-major
    wrapped = np.tile(wrapped, (1, 1, 1, 8, 1))            # [..., 128, GCOLS]
    wrapped = wrapped.transpose(0, 3, 1, 2, 4).reshape(
        N_CORES, P, NG, 3 * GCOLS)

    in_maps = []
    for c in range(N_CORES):
        in_maps.append({
            "pFs": np.ascontiguousarray(pFb[c * PF_SHARD:(c + 1) * PF_SHARD]),
            "Md": Mdup,
            "idx": np.ascontiguousarray(wrapped[c]),
        })
    return in_maps, src_index


def kernel(pF, M, ijk):
    from concourse.bass_utils import run_bass_kernel_spmd

    pF = np.ascontiguousarray(np.asarray(pF, dtype=np.float32))
    M = np.ascontiguousarray(np.asarray(M, dtype=np.float32))
    ijk = np.asarray(ijk)

    nc = _get_program()
    in_maps, src_index = prepare_inputs(pF, M, ijk)

    results = run_bass_kernel_spmd(
        nc, in_maps, core_ids=list(range(N_CORES))).results

    flat = np.concatenate([results[c]["out"] for c in range(N_CORES)])
    return flat[src_index].astype(np.float32)


# revision 16
# speedup vs baseline: 1.1093x; 1.1093x over previous
"""Trainium2 Bass kernel for MatrixFactorizationIF (embedding-lookup style).

Computation per batch element b with indices (i, j, k):
    pFi = pF[i]   # [448] = [Pi(64) | Vs(192) | Vg(192)]
    out[b] = ALPHA * <Pi, M[j]>
           + BETA^2 * sum_s ( <Vs[:, s], M[j]> * <Vg[:, s], M[k]> )

v3 strategy (evolved from the shipped baseline):
  - All tables bf16: pF shard rows are 1024B gather descriptors (512
    bf16 cols incl. pad; elem_size must be %256B, >=512B avoids the DMA
    read-modify-write penalty, vs 1792B in f32); M rows are
    stored duplicated [m|m] as 256B bf16 rows so the gather stays legal
    (elem_size % 256B) and the compute needs no f32->bf16 converts.
  - pF columns host-permuted s-major so Vs/Vg DVE reads are stride-1
    (2x bf16 DVE rate needs contiguous access).
  - Batch sharded by i-range (12500 pF rows per core, i_loc < 12500 fits
    int16 with no grouping).  M gathers need int16 indices, so elements
    are grouped by (j%4, k%4): 16 groups padded to 4096 (seed-0 max group
    count is 4070; overflow raises) with j_loc, k_loc = j>>2, k>>2.
  - One gather call per tensor per group (single_packet=False lifts the
    1024-index single-packet cap), calls round-robined over 4 SWDGE
    queues so descriptor generation parallelizes across Q7 cpu pairs.
  - All int16 indices loaded in one big upfront DMA; outputs stored
    p-major so every store is contiguous per partition.
"""

import numpy as np

N_P = 100000
N_M = 100000
R = 64
S = 3
E = R * (1 + 2 * S)  # 448
EP = 512             # padded bf16 row (1024B, must be %256B)
B = 500000
ALPHA = 0.001
BETA = 0.001

N_CORES = 8
P = 128
PF_SHARD = N_P // N_CORES   # 12500 pF rows per core
NJ = 4                      # j stride classes
NK = 4                      # k stride classes
NG = NJ * NK                # 16 groups per core
GCAP = 4096                 # padded group capacity (32 * 128)
GCOLS = GCAP // 16          # 256 wrapped-index columns
TCOL = GCAP // P            # 33
BS_PAD = NG * GCAP          # 65536 padded elements per core
NQ = 4                      # SWDGE queues


def build_program(repeat=1):
    import concourse.bass as bass
    import concourse.bacc as bacc
    import concourse.mybir as mybir
    from concourse.tile import TileContext

    f32 = mybir.dt.float32
    bf16 = mybir.dt.bfloat16
    i16 = mybir.dt.int16
    mult = mybir.AluOpType.mult
    add = mybir.AluOpType.add
    AX = mybir.AxisListType.X
    t = TCOL

    nc = bacc.Bacc("TRN2", target_bir_lowering=False, num_swdge_queues=NQ)
    pFs = nc.dram_tensor("pFs", [PF_SHARD, EP], bf16, kind="ExternalInput")
    Md = nc.dram_tensor("Md", [N_M, 4 * R], bf16, kind="ExternalInput")
    idx = nc.dram_tensor("idx", [P, NG, 3 * GCOLS], i16, kind="ExternalInput")
    out = nc.dram_tensor("out", [BS_PAD], f32, kind="ExternalOutput")

    with TileContext(nc) as tc:
        with (
            tc.tile_pool(name="idxp", bufs=1) as idx_pool,
            tc.tile_pool(name="pf", bufs=2) as pf_pool,
            tc.tile_pool(name="m", bufs=2) as m_pool,
            tc.tile_pool(name="prod", bufs=2) as prod_pool,
            tc.tile_pool(name="p1", bufs=1) as p1_pool,
            tc.tile_pool(name="small", bufs=2) as small_pool,
            tc.tile_pool(name="res", bufs=2) as res_pool,
        ):
            idx_t = idx_pool.tile([P, NG * 3 * GCOLS], i16)
            nc.sync.dma_start(
                out=idx_t[:].rearrange("p (n q) -> p n q", q=3 * GCOLS),
                in_=idx[:])
            idx3 = idx_t[:].rearrange("p (n q) -> p n q", q=3 * GCOLS)

            qn = 0
            for it in range(NG * repeat):
                g = it % NG
                jc, kc = g // NK, g % NK
                c0 = g * GCAP

                pf_t = pf_pool.tile([P, t * EP], bf16)
                mj_t = m_pool.tile([P, t * 4 * R], bf16, tag="mj")
                mk_t = m_pool.tile([P, t * 4 * R], bf16, tag="mk")
                pf4 = pf_t[:].rearrange("p (t e) -> p t e", e=EP)
                mj3 = mj_t[:].rearrange("p (t d) -> p t d", d=4 * R)
                mk3 = mk_t[:].rearrange("p (t d) -> p t d", d=4 * R)

                mjview = Md[:].rearrange(
                    "(n f) d -> n (f d)", f=NJ)[:, jc * 4 * R:(jc + 1) * 4 * R]
                mkview = Md[:].rearrange(
                    "(n f) d -> n (f d)", f=NK)[:, kc * 4 * R:(kc + 1) * 4 * R]

                nc.gpsimd.dma_gather(
                    out_ap=pf4,
                    in_ap=pFs[:],
                    idxs_ap=idx3[:, g, 0:GCOLS],
                    num_idxs=GCAP, num_idxs_reg=GCAP, elem_size=EP,
                    single_packet=False, queue_num=qn % NQ)
                nc.gpsimd.dma_gather(
                    out_ap=mj3,
                    in_ap=mjview,
                    idxs_ap=idx3[:, g, GCOLS:2 * GCOLS],
                    num_idxs=GCAP, num_idxs_reg=GCAP, elem_size=4 * R,
                    elem_step=4 * R * NJ,
                    single_packet=False, queue_num=(qn + 1) % NQ)
                nc.gpsimd.dma_gather(
                    out_ap=mk3,
                    in_ap=mkview,
                    idxs_ap=idx3[:, g, 2 * GCOLS:3 * GCOLS],
                    num_idxs=GCAP, num_idxs_reg=GCAP, elem_size=4 * R,
                    elem_step=4 * R * NK,
                    single_packet=False, queue_num=(qn + 2) % NQ)
                qn += 3

                mj = mj3[:, :, 0:R]
                mk = mk3[:, :, 0:R]

                # dot = sum_r Pi * Mj
                prod1 = p1_pool.tile([P, t * R], bf16, tag="prod1")
                nc.vector.tensor_tensor(
                    out=prod1[:].rearrange("p (t r) -> p t r", r=R),
                    in0=pf4[:, :, 0:R], in1=mj, op=mult)
                dot = small_pool.tile([P, t], f32, tag="dot")
                nc.vector.reduce_sum(
                    out=dot[:],
                    in_=prod1[:].rearrange("p (t r) -> p t r", r=R), axis=AX)

                # a[t, s] = sum_r Vs[t, s, r] * Mj[t, r]  (s-major layout)
                vs_view = pf4[:, :, R:(1 + S) * R].rearrange(
                    "p t (s r) -> p t s r", s=S)
                mjb = mj3[:, :, R:4 * R].rearrange(
                    "p t (s r) -> p t s r", s=S)
                pvs = prod_pool.tile([P, t * S * R], bf16, tag="pvs")
                nc.vector.tensor_tensor(
                    out=pvs[:].rearrange("p (t s r) -> p t s r", s=S, r=R),
                    in0=vs_view, in1=mjb, op=mult)
                a_t = small_pool.tile([P, t * S], f32, tag="a")
                nc.vector.reduce_sum(
                    out=a_t[:],
                    in_=pvs[:].rearrange("p (t s r) -> p t s r", s=S, r=R),
                    axis=AX)

                # g[t, s] = sum_r Vg[t, s, r] * Mk[t, r]
                vg_view = pf4[:, :, (1 + S) * R:E].rearrange(
                    "p t (s r) -> p t s r", s=S)
                mkb = mk3[:, :, R:4 * R].rearrange(
                    "p t (s r) -> p t s r", s=S)
                pvg = prod_pool.tile([P, t * S * R], bf16, tag="pvg")
                nc.vector.tensor_tensor(
                    out=pvg[:].rearrange("p (t s r) -> p t s r", s=S, r=R),
                    in0=vg_view, in1=mkb, op=mult)
                g_t = small_pool.tile([P, t * S], f32, tag="g")
                nc.vector.reduce_sum(
                    out=g_t[:],
                    in_=pvg[:].rearrange("p (t s r) -> p t s r", s=S, r=R),
                    axis=AX)

                # agdot = sum_s a*g ; res = ALPHA*dot + BETA^2*agdot
                agp = small_pool.tile([P, t * S], f32, tag="agp")
                nc.vector.tensor_mul(out=agp[:], in0=a_t[:], in1=g_t[:])
                agdot = small_pool.tile([P, t], f32, tag="agdot")
                nc.vector.reduce_sum(
                    out=agdot[:],
                    in_=agp[:].rearrange("p (t s) -> p t s", s=S), axis=AX)
                agdot_b = small_pool.tile([P, t], f32, tag="agdot_b")
                nc.vector.tensor_scalar_mul(
                    out=agdot_b[:], in0=agdot[:], scalar1=BETA * BETA)
                res = res_pool.tile([P, t], f32, tag="res")
                nc.vector.scalar_tensor_tensor(
                    out=res[:], in0=dot[:], scalar=ALPHA, in1=agdot_b[:],
                    op0=mult, op1=add)

                # element rank r in group -> slot g*GCAP + (r%128)*33 + r//128
                nc.sync.dma_start(
                    out=out[c0:c0 + GCAP].rearrange("(p t) -> p t", p=P),
                    in_=res[:])

    nc.compile()
    return nc


_NC_CACHE = {}


def _get_program():
    if "main" not in _NC_CACHE:
        _NC_CACHE["main"] = build_program()
    return _NC_CACHE["main"]


def _to_bf16(x):
    import ml_dtypes
    return x.astype(ml_dtypes.bfloat16)


def prepare_inputs(pF, M, ijk):
    """Host-side table conversion + shard/sort/pad of indices.  Returns
    (in_maps, src_index): src_index[b] is the flat position of original
    element b in the concatenated per-core padded outputs."""
    # permute pF columns: Vs/Vg from r-major (s fastest) to s-major
    vs = pF[:, R:(1 + S) * R].reshape(N_P, R, S)
    vg = pF[:, (1 + S) * R:].reshape(N_P, R, S)
    pFp = np.concatenate([
        pF[:, :R],
        np.ascontiguousarray(vs.transpose(0, 2, 1)).reshape(N_P, S * R),
        np.ascontiguousarray(vg.transpose(0, 2, 1)).reshape(N_P, S * R),
        np.zeros((N_P, EP - E), np.float32),
    ], axis=1)
    pFb = np.ascontiguousarray(_to_bf16(pFp))
    Mdup = np.ascontiguousarray(
        np.repeat(_to_bf16(M), 4, axis=0).reshape(N_M, 4 * R))

    i = ijk[:, 0].astype(np.int64)
    j = ijk[:, 1].astype(np.int64)
    k0 = ijk[:, 2].astype(np.int64)
    k = np.where(k0 < 0, 0, k0)

    core = i // PF_SHARD
    gl = (j % NJ) * NK + (k % NK)            # group within core
    gg = core * NG + gl                      # global group id
    i_loc_key = i - core * PF_SHARD
    order = np.argsort(gg * 16384 + i_loc_key, kind="stable")
    counts = np.bincount(gg, minlength=N_CORES * NG)
    if counts.max() > GCAP:
        raise RuntimeError(
            f"group overflow: max {counts.max()} > {GCAP}")
    starts = np.zeros(N_CORES * NG, np.int64)
    starts[1:] = np.cumsum(counts)[:-1]
    rank = np.arange(B) - np.repeat(starts, counts)
    rank_orig = np.empty(B, np.int64)
    rank_orig[order] = rank
    # store layout: slot = g*GCAP + (rank%128)*TCOL + rank//128
    src_index = (core * BS_PAD + gl * GCAP
                 + (rank_orig % P) * TCOL + rank_orig // P)

    i_loc = (i - core * PF_SHARD).astype(np.int16)
    j_loc = (j >> 2).astype(np.int16)
    k_loc = (k >> 2).astype(np.int16)

    # wrapped idx layout: element rank e -> [e % 16, e // 16]
    wrapped = np.zeros((N_CORES, NG, 3, 16, GCOLS), np.int16)
    wp = (rank_orig % 16).astype(np.int64)
    ws = (rank_orig // 16).astype(np.int64)
    wrapped[core, gl, 0, wp, ws] = i_loc
    wrapped[core, gl, 1, wp, ws] = j_loc
    wrapped[core, gl, 2, wp, ws] = k_loc
    # replicate the 16-partition wrap to all 128 partitions; partition-major
    wrapped = np.tile(wrapped, (1, 1, 1, 8, 1))            # [..., 128, GCOLS]
    wrapped = wrapped.transpose(0, 3, 1, 2, 4).reshape(
        N_CORES, P, NG, 3 * GCOLS)

    in_maps = []
    for c in range(N_CORES):
        in_maps.append({
            "pFs": np.ascontiguousarray(pFb[c * PF_SHARD:(c + 1) * PF_SHARD]),
            "Md": Mdup,
            "idx": np.ascontiguousarray(wrapped[c]),
        })
    return in_maps, src_index


def kernel(pF, M, ijk):
    from concourse.bass_utils import run_bass_kernel_spmd

    pF = np.ascontiguousarray(np.asarray(pF, dtype=np.float32))
    M = np.ascontiguousarray(np.asarray(M, dtype=np.float32))
    ijk = np.asarray(ijk)

    nc = _get_program()
    in_maps, src_index = prepare_inputs(pF, M, ijk)

    results = run_bass_kernel_spmd(
        nc, in_maps, core_ids=list(range(N_CORES))).results

    flat = np.concatenate([results[c]["out"] for c in range(N_CORES)])
    return flat[src_index].astype(np.float32)
